# revision 1
# baseline (speedup 1.0000x reference)
import sys

sys.path.insert(0, "/opt/trn_rl_repo")
from contextlib import ExitStack

import numpy as np

import concourse.bass as bass  # noqa: F401
import concourse.mybir as mybir
import concourse.tile as tile
from concourse import bacc
from concourse.bass_utils import run_bass_kernel_spmd

F32 = mybir.dt.float32
F32R = mybir.dt.float32r
AF = mybir.ActivationFunctionType
ALU = mybir.AluOpType
AX = mybir.AxisListType
EPS = 1e-5


def fr(ap):
    return ap.bitcast(F32R)


def build_program():
    nc = bacc.Bacc(trn_type="TRN2")
    d = {}

    def din(name, shape, dt_=F32):
        d[name] = nc.dram_tensor(name, shape, dt_, kind="ExternalInput")
        return d[name]

    xs = din("xs", [8, 3, 224, 224], F32R)
    stem_l = din("stem_l", [36, 384], F32R)
    w1 = din("w1", [32, 2304])
    w2 = din("w2", [64, 4608])
    r1l = din("r1l", [32, 4], F32R)
    r1b = din("r1b", [4, 1])
    r2l = din("r2l", [64, 4], F32R)
    r2b = din("r2b", [4, 1])
    c1bT = din("c1bT", [4, 64], F32R)
    c2bT = din("c2bT", [4, 128], F32R)
    bn0s = din("bn0s", [128, 1])
    bn0h = din("bn0h", [128, 1])
    bn1s64 = din("bn1s64", [64, 1])
    bn1h64 = din("bn1h64", [64, 1])
    bn1s128 = din("bn1s128", [128, 1])
    bn2s = din("bn2s", [128, 1])
    bn2h = din("bn2h", [128, 1])
    ones1 = din("ones1", [1, 128], F32R)
    fcl = din("fcl", [128, 2], F32R)
    fcb = din("fcb", [2, 1])
    zz = din("zz", [128, 1152], F32R)
    out_d = nc.dram_tensor("out", [2, 8], F32, kind="ExternalOutput")

    with tile.TileContext(nc) as tc, ExitStack() as ctx:
        def P(name, bufs, space="SBUF"):
            return ctx.enter_context(tc.tile_pool(name=name, bufs=bufs, space=space))

        consts = P("consts", 1)
        xrp = P("xr", 1)
        h0p = P("h0", 1)
        h1p = P("h1", 1)
        cwpool = P("cw", 2)
        tmpp = P("tmp", 2)
        smallp = P("small", 4)
        gapp = P("gap", 2)
        scrp = P("scr", 1)
        ps_big = P("psb", 2, "PSUM")
        ps_c1 = P("psc1", 3, "PSUM")
        ps_c2 = P("psc2", 2, "PSUM")
        ps_sm = P("pss", 1, "PSUM")

        def lc(dt_, shape, tag, tdt=F32):
            t = consts.tile(shape, tdt, tag=tag)
            nc.sync.dma_start(t[:], dt_[:, :])
            return t

        stem_sb = lc(stem_l, [36, 384], "stem_l", F32R)
        w1sb = lc(w1, [32, 2304], "w1")
        w2sb = lc(w2, [64, 4608], "w2")
        r1lsb = lc(r1l, [32, 4], "r1l", F32R)
        r1bsb = lc(r1b, [4, 1], "r1b")
        r2lsb = lc(r2l, [64, 4], "r2l", F32R)
        r2bsb = lc(r2b, [4, 1], "r2b")
        c1bsb = lc(c1bT, [4, 64], "c1bT", F32R)
        c2bsb = lc(c2bT, [4, 128], "c2bT", F32R)
        bn0ssb = lc(bn0s, [128, 1], "bn0s")
        bn0hsb = lc(bn0h, [128, 1], "bn0h")
        bn1s64sb = lc(bn1s64, [64, 1], "bn1s64")
        bn1h64sb = lc(bn1h64, [64, 1], "bn1h64")
        bn1s128sb = lc(bn1s128, [128, 1], "bn1s128")
        bn2ssb = lc(bn2s, [128, 1], "bn2s")
        bn2hsb = lc(bn2h, [128, 1], "bn2h")
        ones1sb = lc(ones1, [1, 128], "ones1", F32R)
        fclsb = lc(fcl, [128, 2], "fcl", F32R)
        fcbsb = lc(fcb, [2, 1], "fcb")
        zzsb = lc(zz, [128, 1152], "zz", F32R)

        pooled1 = smallp.tile([32, 8], F32R, tag="pooled1")
        pooled2 = smallp.tile([64, 8], F32R, tag="pooled2")
        pooled3 = smallp.tile([128, 8], F32R, tag="pooled3")

        for g in range(2):
            # ---------------- stem for 4 samples of this group ----------------
            h0 = h0p.tile([128, 114 * 114], F32R, tag="h0")
            h03 = h0[:].rearrange("p (r c) -> p r c", c=114)
            nc.sync.dma_start(h03[:, 0:1, :], zzsb[:, 0:114])
            nc.sync.dma_start(h03[:, 113:114, :], zzsb[:, 114:228])
            nc.sync.dma_start(h03[:, :, 0:1], zzsb[:, 228:342])
            nc.sync.dma_start(h03[:, :, 113:114], zzsb[:, 342:456])
            gap0 = gapp.tile([128, 28], F32, tag="gap0")
            for sy in range(14):
                xr = xrp.tile([36, 16 * 226], F32R, tag="xr")
                xr3 = xr[:].rearrange("p (r c) -> p r c", c=226)
                nc.sync.dma_start(xr3[:, :, 0:1], zzsb[0:36, 0:16])
                nc.sync.dma_start(xr3[:, :, 225:226], zzsb[0:36, 16:32])
                for s in range(4):
                    for ky in range(3):
                        r0 = 16 * sy + ky - 1
                        lo = max(r0, 0)
                        hi = min(r0 + 15, 223)
                        nrow = hi - lo + 1
                        dlo = lo - r0
                        base = s * 9 + ky * 3
                        if dlo > 0:
                            nc.sync.dma_start(xr3[base : base + 3, 0:dlo, 1:225],
                                              zzsb[0:3, 0:224])
                        if dlo + nrow < 16:
                            nc.sync.dma_start(
                                xr3[base : base + 3, dlo + nrow : 16, 1:225],
                                zzsb[0:3, 0 : 224 * (16 - dlo - nrow)])
                        nc.sync.dma_start(
                            xr3[base : base + 3, dlo : dlo + nrow, 1:225],
                            xs[g * 4 + s, :, lo : hi + 1, :],
                        )
                for half in range(2):
                    ly0 = half * 4
                    psum = ps_big.tile([128, 448], F32, tag="pstem")
                    for kx in range(3):
                        rhs = xr3[:, 2 * ly0 : 2 * ly0 + 8 : 2, kx : kx + 224 : 2]
                        nc.tensor.matmul(
                            psum[:], fr(stem_sb[:, kx * 128 : kx * 128 + 128]),
                            fr(rhs), start=(kx == 0), stop=(kx == 2),
                        )
                    y0 = 8 * sy + ly0
                    nc.scalar.activation(
                        h03[:, y0 + 1 : y0 + 5, 1:113],
                        psum[:].rearrange("p (r c) -> p r c", c=112),
                        AF.Relu, bias=bn0hsb[:, 0:1], scale=bn0ssb[:, 0:1],
                        accum_out=gap0[:, 2 * sy + half : 2 * sy + half + 1],
                    )
            g1 = smallp.tile([128, 1], F32, tag="g1")
            nc.vector.tensor_reduce(g1[:], gap0[:], AX.X, ALU.add)
            for s in range(4):
                nc.sync.dma_start(
                    pooled1[:, g * 4 + s : g * 4 + s + 1],
                    g1[s * 32 : s * 32 + 32, :].bitcast(F32R),
                )
            # ---------------- routing 1 (4 samples) ----------------
            psr = ps_sm.tile([128, 8], F32, tag="pss")
            nc.tensor.matmul(psr[0:4, 0:4], fr(r1lsb[:]),
                             fr(pooled1[:, g * 4 : g * 4 + 4]), start=True, stop=True)
            r1g = smallp.tile([4, 4], F32R, tag="r1g")
            nc.scalar.activation(r1g[:], psr[0:4, 0:4], AF.Sigmoid,
                                 bias=r1bsb[:, 0:1], scale=1.0)
            rb1 = smallp.tile([128, 16], F32, tag="rb1")
            for k in range(4):
                rrow = smallp.tile([1, 4], F32R, tag="rrow")
                nc.sync.dma_start(rrow[:], r1g[k : k + 1, :])
                psb = ps_sm.tile([128, 8], F32, tag="pss")
                nc.tensor.matmul(psb[:, 0:4], fr(ones1sb[:]), fr(rrow[:]),
                                 start=True, stop=True)
                nc.scalar.activation(rb1[:, k * 4 : k * 4 + 4], psb[:, 0:4],
                                     AF.Identity, bias=0.0, scale=1.0)
            psc = ps_sm.tile([128, 8], F32, tag="pss")
            nc.tensor.matmul(psc[0:64, 0:4], fr(c1bsb[:]), fr(r1g[:]),
                             start=True, stop=True)
            bias1 = smallp.tile([64, 4], F32, tag="bias1")
            nc.scalar.activation(bias1[:], psc[0:64, 0:4], AF.Identity,
                                 bias=bn1h64sb[:, 0:1], scale=bn1s64sb[:, 0:1])

            cwt = cwpool.tile([128, 1152], F32R, tag="cw1")
            cwt3z = cwt[:].rearrange("p (t m) -> p t m", m=128)
            for zpr in range(2):
                for zsl in range(2):
                    nc.sync.dma_start(
                        cwt3z[zpr * 64 + zsl * 32 : zpr * 64 + zsl * 32 + 32, :,
                              (1 - zsl) * 64 : (1 - zsl) * 64 + 64],
                        zzsb[0:32, 0:576])
            for pr in range(2):
                # ---------------- conv1 for sample pair ----------------
                for sl in range(2):
                    sloc = pr * 2 + sl
                    acc = tmpp.tile([32, 576], F32, tag="t1a")
                    tmpb = tmpp.tile([32, 576], F32, tag="t1b")
                    nc.scalar.activation(acc[:], w1sb[:, 0:576], AF.Copy,
                                         scale=rb1[0:32, sloc : sloc + 1])
                    nc.scalar.activation(tmpb[:], w1sb[:, 576:1152], AF.Copy,
                                         scale=rb1[0:32, 4 + sloc : 5 + sloc])
                    nc.vector.tensor_add(acc[:], acc[:], tmpb[:])
                    tmpb = tmpp.tile([32, 576], F32, tag="t1b")
                    nc.scalar.activation(tmpb[:], w1sb[:, 1152:1728], AF.Copy,
                                         scale=rb1[0:32, 8 + sloc : 9 + sloc])
                    nc.vector.tensor_add(acc[:], acc[:], tmpb[:])
                    tmpb = tmpp.tile([32, 576], F32, tag="t1b")
                    nc.scalar.activation(tmpb[:], w1sb[:, 1728:2304], AF.Copy,
                                         scale=rb1[0:32, 12 + sloc : 13 + sloc])
                    nc.vector.tensor_add(acc[:], acc[:], tmpb[:])
                    nc.sync.dma_start(
                        cwt[:].rearrange("p (t m) -> p t m", m=128)
                           [pr * 64 + sl * 32 : pr * 64 + sl * 32 + 32, :,
                            sl * 64 : sl * 64 + 64],
                        acc[:].bitcast(F32R).rearrange("p (t m) -> p t m", m=64),
                    )
                pbias = smallp.tile([128, 1], F32, tag="pbias")
                nc.sync.dma_start(pbias[0:64, :], bias1[:, pr * 2 : pr * 2 + 1])
                nc.sync.dma_start(pbias[64:128, :], bias1[:, pr * 2 + 1 : pr * 2 + 2])
                h1 = h1p.tile([128, 114 * 114], F32R, tag="h1")
                h13 = h1[:].rearrange("p (r c) -> p r c", c=114)
                nc.sync.dma_start(h13[:, 0:1, :], zzsb[:, 0:114])
                nc.sync.dma_start(h13[:, 113:114, :], zzsb[:, 114:228])
                nc.sync.dma_start(h13[:, :, 0:1], zzsb[:, 228:342])
                nc.sync.dma_start(h13[:, :, 113:114], zzsb[:, 342:456])
                gap1 = gapp.tile([128, 28], F32, tag="gap1")
                for ch in range(28):
                    y0 = 4 * ch
                    psum = ps_c1.tile([128, 448], F32, tag="pc1")
                    for t in range(9):
                        ky, kx = t // 3, t % 3
                        rhs = h03[pr * 64 : pr * 64 + 64,
                                  y0 + ky : y0 + ky + 4, kx : kx + 112]
                        nc.tensor.matmul(
                            psum[:], fr(cwt[pr * 64 : pr * 64 + 64, t * 128 : t * 128 + 128]),
                            fr(rhs), start=(t == 0), stop=(t == 8),
                        )
                    nc.scalar.activation(
                        h13[:, y0 + 1 : y0 + 5, 1:113],
                        psum[:].rearrange("p (r c) -> p r c", c=112),
                        AF.Relu, bias=pbias[:, 0:1], scale=bn1s128sb[:, 0:1],
                        accum_out=gap1[:, ch : ch + 1],
                    )
                g2 = smallp.tile([128, 1], F32, tag="g2")
                nc.vector.tensor_reduce(g2[:], gap1[:], AX.X, ALU.add)
                col0 = g * 4 + pr * 2
                nc.sync.dma_start(pooled2[:, col0 : col0 + 1], g2[0:64, :].bitcast(F32R))
                nc.sync.dma_start(pooled2[:, col0 + 1 : col0 + 2], g2[64:128, :].bitcast(F32R))
                # ---------------- routing 2 (pair) ----------------
                ps2 = ps_sm.tile([128, 8], F32, tag="pss")
                nc.tensor.matmul(ps2[0:4, 0:2], fr(r2lsb[:]),
                                 fr(pooled2[:, col0 : col0 + 2]), start=True, stop=True)
                r2g = smallp.tile([4, 2], F32R, tag="r2g")
                nc.scalar.activation(r2g[:], ps2[0:4, 0:2], AF.Sigmoid,
                                     bias=r2bsb[:, 0:1], scale=1.0)
                rb2 = smallp.tile([128, 8], F32, tag="rb2")
                for k in range(4):
                    rrow2 = smallp.tile([1, 2], F32R, tag="rrow2")
                    nc.sync.dma_start(rrow2[:], r2g[k : k + 1, :])
                    psb2 = ps_sm.tile([128, 8], F32, tag="pss")
                    nc.tensor.matmul(psb2[:, 0:2], fr(ones1sb[:]), fr(rrow2[:]),
                                     start=True, stop=True)
                    nc.scalar.activation(rb2[:, k * 2 : k * 2 + 2], psb2[:, 0:2],
                                         AF.Identity, bias=0.0, scale=1.0)
                psc2 = ps_sm.tile([128, 8], F32, tag="pss")
                nc.tensor.matmul(psc2[:, 0:2], fr(c2bsb[:]), fr(r2g[:]),
                                 start=True, stop=True)
                bias2 = smallp.tile([128, 2], F32, tag="bias2")
                nc.scalar.activation(bias2[:], psc2[:, 0:2], AF.Identity,
                                     bias=bn2hsb[:, 0:1], scale=bn2ssb[:, 0:1])
                # ---------------- conv2 per sample ----------------
                cw2f = cwpool.tile([128, 1152], F32R, tag="cw2")
                for sl in range(2):
                    cw2 = tmpp.tile([64, 1152], F32, tag="t2a")
                    tm = tmpp.tile([64, 1152], F32, tag="t2b")
                    nc.scalar.activation(cw2[:], w2sb[:, 0:1152], AF.Copy,
                                         scale=rb2[0:64, sl : sl + 1])
                    nc.scalar.activation(tm[:], w2sb[:, 1152:2304], AF.Copy,
                                         scale=rb2[0:64, 2 + sl : 3 + sl])
                    nc.vector.tensor_add(cw2[:], cw2[:], tm[:])
                    tm = tmpp.tile([64, 1152], F32, tag="t2b")
                    nc.scalar.activation(tm[:], w2sb[:, 2304:3456], AF.Copy,
                                         scale=rb2[0:64, 4 + sl : 5 + sl])
                    nc.vector.tensor_add(cw2[:], cw2[:], tm[:])
                    tm = tmpp.tile([64, 1152], F32, tag="t2b")
                    nc.scalar.activation(tm[:], w2sb[:, 3456:4608], AF.Copy,
                                         scale=rb2[0:64, 6 + sl : 7 + sl])
                    nc.vector.tensor_add(cw2[:], cw2[:], tm[:])
                    nc.sync.dma_start(cw2f[sl * 64 : sl * 64 + 64, :], cw2[:].bitcast(F32R))
                    gap2 = gapp.tile([128, 7], F32, tag="gap2")
                    for ch in range(7):
                        y0 = 8 * ch
                        psum = ps_c2.tile([128, 448], F32, tag="pc2")
                        for t in range(9):
                            ky, kx = t // 3, t % 3
                            rhs = h13[sl * 64 : sl * 64 + 64,
                                      2 * y0 + ky : 2 * y0 + ky + 16 : 2,
                                      kx : kx + 112 : 2]
                            nc.tensor.matmul(
                                psum[:], fr(cw2f[sl * 64 : sl * 64 + 64, t * 128 : t * 128 + 128]),
                                fr(rhs), start=(t == 0), stop=(t == 8),
                            )
                        scr = scrp.tile([128, 448], F32, tag="scr")
                        nc.scalar.activation(
                            scr[:], psum[:], AF.Relu,
                            bias=bias2[:, sl : sl + 1], scale=bn2ssb[:, 0:1],
                            accum_out=gap2[:, ch : ch + 1],
                        )
                    scol = g * 4 + pr * 2 + sl
                    g3 = smallp.tile([128, 1], F32, tag="g3")
                    nc.vector.tensor_reduce(g3[:], gap2[:], AX.X, ALU.add)
                    nc.sync.dma_start(pooled3[:, scol : scol + 1],
                                      g3[:].bitcast(F32R))
        # ---------------- head ----------------
        psf = ps_sm.tile([128, 8], F32, tag="pss")
        nc.tensor.matmul(psf[0:2, 0:8], fr(fclsb[:]), fr(pooled3[:]),
                         start=True, stop=True)
        outsb = smallp.tile([2, 8], F32, tag="outsb")
        nc.scalar.activation(outsb[:], psf[0:2, 0:8], AF.Identity,
                             bias=fcbsb[:, 0:1], scale=1.0)
        nc.sync.dma_start(out_d[:, :], outsb[:])
    nc.finalize()
    return nc


def prep_consts(i):
    def bn(g, b, m, v):
        sc = g / np.sqrt(v + EPS)
        return sc.astype(np.float32), (b - m * sc).astype(np.float32)

    c = {}
    sw = np.asarray(i["stem_w"], np.float32)
    t = sw.transpose(2, 1, 0, 3).reshape(9, 32, 3)  # [(ky,c), co, kx]
    stem_l = np.zeros((36, 384), np.float32)
    for s in range(4):
        for kx in range(3):
            stem_l[s * 9 : s * 9 + 9, kx * 128 + s * 32 : kx * 128 + s * 32 + 32] = t[:, :, kx]
    c["stem_l"] = stem_l
    c["w1"] = np.ascontiguousarray(
        np.asarray(i["c1_w"], np.float32).transpose(0, 3, 4, 1, 2).reshape(4 * 9 * 64, 32).T)
    c["w2"] = np.ascontiguousarray(
        np.asarray(i["c2_w"], np.float32).transpose(0, 3, 4, 1, 2).reshape(4 * 9 * 128, 64).T)
    c["r1l"] = np.ascontiguousarray((np.asarray(i["r1_w"], np.float32) / 12544.0).T)
    c["r1b"] = np.asarray(i["r1_b"], np.float32).reshape(4, 1)
    c["r2l"] = np.ascontiguousarray((np.asarray(i["r2_w"], np.float32) / 12544.0).T)
    c["r2b"] = np.asarray(i["r2_b"], np.float32).reshape(4, 1)
    c["c1bT"] = np.asarray(i["c1_b"], np.float32)
    c["c2bT"] = np.asarray(i["c2_b"], np.float32)
    s0, h0 = bn(*[np.asarray(i[k], np.float32) for k in ("bn0_g", "bn0_b", "bn0_m", "bn0_v")])
    c["bn0s"] = np.tile(s0, 4).reshape(128, 1)
    c["bn0h"] = np.tile(h0, 4).reshape(128, 1)
    s1, h1 = bn(*[np.asarray(i[k], np.float32) for k in ("bn1_g", "bn1_b", "bn1_m", "bn1_v")])
    c["bn1s64"] = s1.reshape(64, 1)
    c["bn1h64"] = h1.reshape(64, 1)
    c["bn1s128"] = np.tile(s1, 2).reshape(128, 1)
    s2, h2 = bn(*[np.asarray(i[k], np.float32) for k in ("bn2_g", "bn2_b", "bn2_m", "bn2_v")])
    c["bn2s"] = s2.reshape(128, 1)
    c["bn2h"] = h2.reshape(128, 1)
    c["ones1"] = np.ones((1, 128), np.float32)
    c["fcl"] = np.ascontiguousarray((np.asarray(i["fc_w"], np.float32) / 3136.0).T)
    c["fcb"] = np.asarray(i["fc_b"], np.float32).reshape(2, 1)
    c["zz"] = np.zeros((128, 1152), np.float32)
    return c


_PROG = None


def kernel(**inputs):
    global _PROG
    if _PROG is None:
        _PROG = build_program()
    nc = _PROG
    c = prep_consts(inputs)
    x = np.ascontiguousarray(np.asarray(inputs["x"], np.float32))
    in_maps = []
    for core in range(8):
        m = dict(c)
        m["xs"] = np.ascontiguousarray(x[core * 8 : core * 8 + 8])
        in_maps.append(m)
    res = run_bass_kernel_spmd(nc, in_maps, core_ids=list(range(8)))
    out = np.concatenate([r["out"].T for r in res.results], axis=0)
    return out.astype(np.float32)



# revision 14
# speedup vs baseline: 1.5967x; 1.5967x over previous
import sys

sys.path.insert(0, "/opt/trn_rl_repo")
from contextlib import ExitStack

import numpy as np

import concourse.bass as bass  # noqa: F401
import concourse.mybir as mybir
import concourse.tile as tile
from concourse import bacc
from concourse.bass_utils import run_bass_kernel_spmd

F32 = mybir.dt.float32
F32R = mybir.dt.float32r
BF16 = mybir.dt.bfloat16
AF = mybir.ActivationFunctionType
ALU = mybir.AluOpType
AX = mybir.AxisListType
EPS = 1e-5

NP_BF16 = mybir.dt.np(BF16)


def fr(ap):
    return ap.bitcast(F32R)


def build_program():
    nc = bacc.Bacc(trn_type="TRN2")

    def din(name, shape, dt_=F32):
        return nc.dram_tensor(name, shape, dt_, kind="ExternalInput")

    xs = din("xs", [8, 3, 224, 224], BF16)
    stem_l = din("stem_l", [128, 384], BF16)
    w1e_d = din("w1e", [128, 576], BF16)
    w2ea_d = din("w2ea", [128, 1152], BF16)
    w2eb_d = din("w2eb", [128, 1152], BF16)
    bid1_d = din("bid1", [128, 32], BF16)
    bid2_d = din("bid2", [128, 64], BF16)
    e1x_d = din("e1x", [4, 128], F32R)
    e2a_d = din("e2a", [4, 128], F32R)
    e2b_d = din("e2b", [4, 128], F32R)
    r1l_d = din("r1l", [32, 4], F32R)
    r1b_d = din("r1b", [4, 1])
    r2l_d = din("r2l", [64, 4], F32R)
    r2b_d = din("r2b", [4, 1])
    c1bT_d = din("c1bT", [4, 64], F32R)
    bn1h_d = din("bn1h64", [64, 1])
    c2bT_d = din("c2bT", [4, 128], F32R)
    bn2h_d = din("bn2h", [128, 1])
    bn0h_d = din("bn0h", [128, 1])
    fcl_d = din("fcl", [128, 2], F32R)
    fcb_d = din("fcb", [2, 1])
    out_d = nc.dram_tensor("out", [2, 8], F32, kind="ExternalOutput")

    with tile.TileContext(nc) as tc, ExitStack() as ctx:
        def P(name, bufs, space="SBUF"):
            return ctx.enter_context(tc.tile_pool(name=name, bufs=bufs, space=space))

        consts = P("consts", 1)
        xrp = P("xr", 1)
        h0p = P("h0", 1)
        h1p = P("h1", 2)
        cw1p = P("cw1", 2)
        cw2p = P("cw2", 2)
        sc1p = P("sc1", 2)
        sc2p = P("sc2", 2)
        scrp = P("scr", 2)
        smallp = P("small", 4)
        gapp = P("gap", 2)
        ps = P("ps", 4, "PSUM")

        def lc(dt_, shape, tag, tdt=F32):
            t = consts.tile(shape, tdt, tag=tag)
            # separate queue from the strided bf16 xs loads: interleaving
            # small f32 DMAs with them on one DGE queue corrupts transfers
            nc.scalar.dma_start(t[:], dt_[:, :])
            return t

        stem_sb = lc(stem_l, [128, 384], "stem_l", BF16)
        w1e = lc(w1e_d, [128, 576], "w1e", BF16)
        w2ea = lc(w2ea_d, [128, 1152], "w2ea", BF16)
        w2eb = lc(w2eb_d, [128, 1152], "w2eb", BF16)
        bid1 = lc(bid1_d, [128, 32], "bid1", BF16)
        bid2 = lc(bid2_d, [128, 64], "bid2", BF16)
        e1x = lc(e1x_d, [4, 128], "e1x", F32R)
        e2a = lc(e2a_d, [4, 128], "e2a", F32R)
        e2b = lc(e2b_d, [4, 128], "e2b", F32R)
        r1l = lc(r1l_d, [32, 4], "r1l", F32R)
        r1b = lc(r1b_d, [4, 1], "r1b")
        r2l = lc(r2l_d, [64, 4], "r2l", F32R)
        r2b = lc(r2b_d, [4, 1], "r2b")
        c1bT = lc(c1bT_d, [4, 64], "c1bT", F32R)
        bn1h = lc(bn1h_d, [64, 1], "bn1h")
        c2bT = lc(c2bT_d, [4, 128], "c2bT", F32R)
        bn2h = lc(bn2h_d, [128, 1], "bn2h")
        bn0h = lc(bn0h_d, [128, 1], "bn0h")
        fcl = lc(fcl_d, [128, 2], "fcl", F32R)
        fcb = lc(fcb_d, [2, 1], "fcb")

        # ---- x preload: xr[64g + 9s + 3ky + c, R, C] = x[4g+s, c, 2R-1+ky, C-1]
        # only even input-row phase is ever read by the stride-2 stem.
        xr = xrp.tile([100, 112 * 228], BF16, tag="xr")
        xr3 = xr[:].rearrange("p (r c) -> p r c", c=228)
        nc.gpsimd.memset(xr3[0:100, :, 0:2], 0.0)
        # row R=0 zero for ky=0 partitions; ky=1/2 loads overwrite it below
        nc.gpsimd.memset(xr3[0:100, 0:1, :], 0.0)
        for g in range(2):
            for s in range(4):
                base = 64 * g + 9 * s
                # ky = 0: rows 1,3,...,221 -> R 1..111 ; R=0 is zero pad
                nc.sync.dma_start(
                    xr3[base : base + 3, 1:112, 2:226],
                    xs[4 * g + s, :, 1:222:2, :],
                )
                # ky = 1: rows 0,2,...,222 -> R 0..111
                nc.sync.dma_start(
                    xr3[base + 3 : base + 6, 0:112, 2:226],
                    xs[4 * g + s, :, 0:223:2, :],
                )
                # ky = 2: rows 1,3,...,223 -> R 0..111
                nc.sync.dma_start(
                    xr3[base + 6 : base + 9, 0:112, 2:226],
                    xs[4 * g + s, :, 1:224:2, :],
                )

        h0 = h0p.tile([128, 114 * 114], BF16, tag="h0")
        h03 = h0[:].rearrange("p (r c) -> p r c", c=114)
        nc.gpsimd.memset(h03[:, 0:1, :], 0.0)
        nc.gpsimd.memset(h03[:, 113:114, :], 0.0)
        nc.gpsimd.memset(h03[:, :, 0:1], 0.0)
        nc.gpsimd.memset(h03[:, :, 113:114], 0.0)

        pooled1 = smallp.tile([32, 8], F32R, tag="pooled1")
        pooled2 = smallp.tile([64, 8], F32R, tag="pooled2")
        pooled3 = smallp.tile([128, 8], F32R, tag="pooled3")

        for g in range(2):
            # ---------------- stem ----------------
            gap0 = gapp.tile([128, 14], F32, tag="gap0")
            for sy in range(14):
                pst = ps.tile([128, 1024], F32, tag="pb")
                for c2 in range(2):
                    y0 = 8 * sy + 4 * c2
                    for kx in range(3):
                        nc.tensor.matmul(
                            pst[:, 512 * c2 : 512 * c2 + 448],
                            stem_sb[64 * g : 64 * g + 36, kx * 128 : kx * 128 + 128],
                            xr3[64 * g : 64 * g + 36, y0 : y0 + 4, kx + 1 : kx + 224 : 2],
                            start=(kx == 0),
                            stop=(kx == 2),
                        )
                nc.scalar.activation(
                    h03[:, 8 * sy + 1 : 8 * sy + 9, 1:113].rearrange(
                        "p (a b) c -> p a b c", a=2
                    ),
                    pst[:, 0:1024].rearrange("p (a b) -> p a b", a=2)[:, :, 0:448]
                    .rearrange("p a (b c) -> p a b c", b=4),
                    AF.Relu,
                    bias=bn0h[:, 0:1],
                    scale=1.0,
                    accum_out=gap0[:, sy : sy + 1],
                )
            g1 = smallp.tile([128, 1], F32, tag="g1")
            nc.vector.tensor_reduce(g1[:], gap0[:], AX.X, ALU.add)
            for s in range(4):
                nc.sync.dma_start(
                    pooled1[:, 4 * g + s : 4 * g + s + 1],
                    g1[32 * s : 32 * s + 32, :].bitcast(F32R),
                )
            # ---------------- routing 1 ----------------
            psr = ps.tile([128, 1024], F32, tag="pb")
            nc.tensor.matmul(
                psr[0:4, 0:4], fr(r1l[:]), fr(pooled1[:, 4 * g : 4 * g + 4]),
                start=True, stop=True,
            )
            r1g = smallp.tile([4, 4], F32R, tag="r1g")
            nc.scalar.activation(r1g[:], psr[0:4, 0:4], AF.Sigmoid,
                                 bias=r1b[:, 0:1], scale=1.0)
            # broadcast r over (k, c) partitions: r1full[32k+c, s] = r1g[k, s]
            psb = ps.tile([128, 1024], F32, tag="pb")
            nc.tensor.matmul(psb[:, 0:4], fr(e1x[:]), fr(r1g[:]), start=True, stop=True)
            r1full = smallp.tile([128, 4], F32, tag="r1full")
            nc.scalar.copy(r1full[:], psb[:, 0:4])
            # bias1[co, s] = (r . c1b*s1)[co, s] + bn1h[co]
            psc = ps.tile([128, 1024], F32, tag="pb")
            nc.tensor.matmul(psc[0:64, 0:4], fr(c1bT[:]), fr(r1g[:]), start=True, stop=True)
            bias1 = smallp.tile([64, 4], F32, tag="bias1")
            nc.scalar.activation(bias1[:], psc[0:64, 0:4], AF.Identity,
                                 bias=bn1h[:, 0:1], scale=1.0)
            # ---------------- combine conv1 weights ----------------
            # cwt[64pr + 32sl + c, (64sl+co)*9 + t] = sum_k r[j,k] W1[k,co,c,t]
            cwt = cw1p.tile([128, 1152], BF16, tag="cwt")
            for pr in range(2):
                for sl in range(2):
                    nc.gpsimd.memset(
                        cwt[64 * pr + 32 * sl : 64 * pr + 32 * sl + 32,
                            (1 - sl) * 576 : (2 - sl) * 576], 0.0)
            psw = ps.tile([128, 1024], F32, tag="pb")
            for j in range(4):
                tj = sc1p.tile([128, 576], BF16, tag="sc1")
                nc.vector.tensor_scalar(tj[:], w1e[:], r1full[:, j : j + 1], None, ALU.mult)
                for h in range(2):
                    nc.tensor.matmul(
                        psw[32 * j : 32 * j + 32, 512 * h : 512 * h + 288],
                        bid1[:], tj[:, 288 * h : 288 * h + 288],
                        start=True, stop=True, tile_position=(0, 32 * j),
                    )
            for pr in range(2):
                for sl in range(2):
                    j = 2 * pr + sl
                    nc.scalar.copy(
                        cwt[64 * pr + 32 * sl : 64 * pr + 32 * sl + 32,
                            576 * sl : 576 * sl + 576].rearrange(
                                "p (a b) -> p a b", a=2),
                        psw[32 * j : 32 * j + 32, 0:1024].rearrange(
                            "p (a b) -> p a b", a=2)[:, :, 0:288],
                    )
            for pr in range(2):
                # ---------------- conv1 for sample pair ----------------
                pbias = smallp.tile([128, 1], F32, tag="pbias")
                nc.sync.dma_start(pbias[0:64, :], bias1[:, 2 * pr : 2 * pr + 1])
                nc.sync.dma_start(pbias[64:128, :], bias1[:, 2 * pr + 1 : 2 * pr + 2])
                h1 = h1p.tile([128, 114 * 114], BF16, tag="h1")
                h13 = h1[:].rearrange("p (r c) -> p r c", c=114)
                nc.gpsimd.memset(h13[:, 0:1, :], 0.0)
                nc.gpsimd.memset(h13[:, 113:114, :], 0.0)
                nc.gpsimd.memset(h13[:, :, 0:1], 0.0)
                nc.gpsimd.memset(h13[:, :, 113:114], 0.0)
                gap1 = gapp.tile([128, 14], F32, tag="gap1")
                for grp in range(14):
                    pst = ps.tile([128, 1024], F32, tag="pb")
                    for c2 in range(2):
                        y0 = 8 * grp + 4 * c2
                        for t in range(9):
                            ky, kx = t // 3, t % 3
                            nc.tensor.matmul(
                                pst[:, 512 * c2 : 512 * c2 + 448],
                                cwt[64 * pr : 64 * pr + 64, t : 1152 : 9],
                                h03[64 * pr : 64 * pr + 64,
                                    y0 + ky : y0 + ky + 4, kx : kx + 112],
                                start=(t == 0),
                                stop=(t == 8),
                            )
                    nc.scalar.activation(
                        h13[:, 8 * grp + 1 : 8 * grp + 9, 1:113].rearrange(
                            "p (a b) c -> p a b c", a=2),
                        pst[:, 0:1024].rearrange("p (a b) -> p a b", a=2)[:, :, 0:448]
                        .rearrange("p a (b c) -> p a b c", b=4),
                        AF.Relu,
                        bias=pbias[:, 0:1],
                        scale=1.0,
                        accum_out=gap1[:, grp : grp + 1],
                    )
                g2 = smallp.tile([128, 1], F32, tag="g2")
                nc.vector.tensor_reduce(g2[:], gap1[:], AX.X, ALU.add)
                col0 = 4 * g + 2 * pr
                nc.sync.dma_start(pooled2[:, col0 : col0 + 1], g2[0:64, :].bitcast(F32R))
                nc.sync.dma_start(pooled2[:, col0 + 1 : col0 + 2], g2[64:128, :].bitcast(F32R))
                # ---------------- routing 2 ----------------
                ps2 = ps.tile([128, 1024], F32, tag="pb")
                nc.tensor.matmul(ps2[0:4, 0:2], fr(r2l[:]),
                                 fr(pooled2[:, col0 : col0 + 2]), start=True, stop=True)
                r2g = smallp.tile([4, 2], F32R, tag="r2g")
                nc.scalar.activation(r2g[:], ps2[0:4, 0:2], AF.Sigmoid,
                                     bias=r2b[:, 0:1], scale=1.0)
                # r2full[64j+c, 2*tau+s] = r2g[2*tau+j, s]
                psb2 = ps.tile([128, 1024], F32, tag="pb")
                nc.tensor.matmul(psb2[:, 0:2], fr(e2a[:]), fr(r2g[:]), start=True, stop=True)
                nc.tensor.matmul(psb2[:, 512:514], fr(e2b[:]), fr(r2g[:]), start=True, stop=True)
                r2full = smallp.tile([128, 4], F32, tag="r2full")
                nc.scalar.copy(
                    r2full[:].rearrange("p (a b) -> p a b", a=2),
                    psb2[:, 0:1024].rearrange("p (a b) -> p a b", a=2)[:, :, 0:2],
                )
                # bias2[co', s] with bn2h
                psc2 = ps.tile([128, 1024], F32, tag="pb")
                nc.tensor.matmul(psc2[:, 0:2], fr(c2bT[:]), fr(r2g[:]), start=True, stop=True)
                bias2 = smallp.tile([128, 2], F32, tag="bias2")
                nc.scalar.activation(bias2[:], psc2[:, 0:2], AF.Identity,
                                     bias=bn2h[:, 0:1], scale=1.0)
                for sl in range(2):
                    # ---------------- combine conv2 weights ----------------
                    cw2f = cw2p.tile([128, 1152], BF16, tag="cw2f")
                    ta = sc2p.tile([128, 1152], BF16, tag="sc2")
                    nc.vector.tensor_scalar(ta[:], w2ea[:], r2full[:, sl : sl + 1],
                                            None, ALU.mult)
                    tb = sc2p.tile([128, 1152], BF16, tag="sc2")
                    nc.vector.tensor_scalar(tb[:], w2eb[:], r2full[:, 2 + sl : 3 + sl],
                                            None, ALU.mult)
                    for half in range(2):
                        psw2 = ps.tile([128, 1024], F32, tag="pb")
                        for q in range(2):
                            qq = 2 * half + q
                            nc.tensor.matmul(
                                psw2[64 * sl : 64 * sl + 64, 512 * q : 512 * q + 288],
                                bid2[:], ta[:, 288 * qq : 288 * qq + 288],
                                start=True, stop=False, tile_position=(0, 64 * sl),
                            )
                            nc.tensor.matmul(
                                psw2[64 * sl : 64 * sl + 64, 512 * q : 512 * q + 288],
                                bid2[:], tb[:, 288 * qq : 288 * qq + 288],
                                start=False, stop=True, tile_position=(0, 64 * sl),
                            )
                        nc.scalar.copy(
                            cw2f[64 * sl : 64 * sl + 64,
                                 576 * half : 576 * half + 576].rearrange(
                                     "p (a b) -> p a b", a=2),
                            psw2[64 * sl : 64 * sl + 64, 0:1024].rearrange(
                                "p (a b) -> p a b", a=2)[:, :, 0:288],
                        )
                    # ---------------- conv2 ----------------
                    gap2 = gapp.tile([128, 4], F32, tag="gap2")
                    for grp in range(4):
                        nch = 2 if grp < 3 else 1
                        pst = ps.tile([128, 1024], F32, tag="pb")
                        for c2 in range(nch):
                            y0 = 16 * (2 * grp + c2)
                            for t in range(9):
                                ky, kx = t // 3, t % 3
                                nc.tensor.matmul(
                                    pst[:, 512 * c2 : 512 * c2 + 448],
                                    cw2f[64 * sl : 64 * sl + 64, t : 1152 : 9],
                                    h13[64 * sl : 64 * sl + 64,
                                        y0 + ky : y0 + ky + 16 : 2,
                                        kx : kx + 112 : 2],
                                    start=(t == 0),
                                    stop=(t == 8),
                                )
                        scr = scrp.tile([128, 896], BF16, tag="scr")
                        if nch == 2:
                            nc.scalar.activation(
                                scr[:].rearrange("p (a b) -> p a b", a=2),
                                pst[:, 0:1024].rearrange("p (a b) -> p a b", a=2)
                                [:, :, 0:448],
                                AF.Relu,
                                bias=bias2[:, sl : sl + 1],
                                scale=1.0,
                                accum_out=gap2[:, grp : grp + 1],
                            )
                        else:
                            nc.scalar.activation(
                                scr[:, 0:448],
                                pst[:, 0:448],
                                AF.Relu,
                                bias=bias2[:, sl : sl + 1],
                                scale=1.0,
                                accum_out=gap2[:, grp : grp + 1],
                            )
                    g3 = smallp.tile([128, 1], F32, tag="g3")
                    nc.vector.tensor_reduce(g3[:], gap2[:], AX.X, ALU.add)
                    scol = 4 * g + 2 * pr + sl
                    nc.sync.dma_start(pooled3[:, scol : scol + 1], g3[:].bitcast(F32R))
        # ---------------- head ----------------
        psf = ps.tile([128, 1024], F32, tag="pb")
        nc.tensor.matmul(psf[0:2, 0:8], fr(fcl[:]), fr(pooled3[:]), start=True, stop=True)
        outsb = smallp.tile([2, 8], F32, tag="outsb")
        nc.scalar.activation(outsb[:], psf[0:2, 0:8], AF.Identity,
                             bias=fcb[:, 0:1], scale=1.0)
        nc.sync.dma_start(out_d[:, :], outsb[:])
    nc.finalize()
    return nc


def prep_consts(i):
    def bn(g, b, m, v):
        sc = g / np.sqrt(v + EPS)
        return sc.astype(np.float32), (b - m * sc).astype(np.float32)

    c = {}
    s0, h0v = bn(*[np.asarray(i[k], np.float32) for k in ("bn0_g", "bn0_b", "bn0_m", "bn0_v")])
    s1, h1v = bn(*[np.asarray(i[k], np.float32) for k in ("bn1_g", "bn1_b", "bn1_m", "bn1_v")])
    s2, h2v = bn(*[np.asarray(i[k], np.float32) for k in ("bn2_g", "bn2_b", "bn2_m", "bn2_v")])

    # stem_l[64g + 9s + 3ky + c, 128kx + 32s + co] = W[co,c,ky,kx]*s0[co]
    sw = np.asarray(i["stem_w"], np.float32) * s0[:, None, None, None]  # [32,3,3,3]
    base = sw.transpose(2, 1, 3, 0)  # [ky, c, kx, co]
    stem_l = np.zeros((128, 384), np.float32)
    for g in range(2):
        for s in range(4):
            for ky in range(3):
                for cc in range(3):
                    for kx in range(3):
                        stem_l[64 * g + 9 * s + 3 * ky + cc,
                               128 * kx + 32 * s : 128 * kx + 32 * s + 32] = base[ky, cc, kx]
    c["stem_l"] = stem_l.astype(NP_BF16)

    # w1e[32k + c, 9co + 3ky + kx] = W1[k, co, c, ky, kx]*s1[co]
    w1 = np.asarray(i["c1_w"], np.float32) * s1[None, :, None, None, None]  # [4,64,32,3,3]
    w1e = w1.transpose(0, 2, 1, 3, 4).reshape(4, 32, 64 * 9).reshape(128, 576)
    c["w1e"] = np.ascontiguousarray(w1e).astype(NP_BF16)

    # w2e_tau[64k' + c, 9co + t] = W2[2tau + k', co, c, t]*s2[co]
    w2 = np.asarray(i["c2_w"], np.float32) * s2[None, :, None, None, None]  # [4,128,64,3,3]
    w2p = w2.transpose(0, 2, 1, 3, 4).reshape(4, 64, 128 * 9)
    c["w2ea"] = np.ascontiguousarray(w2p[0:2].reshape(128, 1152)).astype(NP_BF16)
    c["w2eb"] = np.ascontiguousarray(w2p[2:4].reshape(128, 1152)).astype(NP_BF16)

    c["bid1"] = np.tile(np.eye(32, dtype=np.float32), (4, 1)).astype(NP_BF16)
    c["bid2"] = np.tile(np.eye(64, dtype=np.float32), (2, 1)).astype(NP_BF16)
    c["e1x"] = np.repeat(np.eye(4, dtype=np.float32), 32, axis=1)
    c["e2a"] = np.repeat(np.eye(4, dtype=np.float32)[:, 0:2], 64, axis=1)
    c["e2b"] = np.repeat(np.eye(4, dtype=np.float32)[:, 2:4], 64, axis=1)

    c["r1l"] = np.ascontiguousarray((np.asarray(i["r1_w"], np.float32) / 12544.0).T)
    c["r1b"] = np.asarray(i["r1_b"], np.float32).reshape(4, 1)
    c["r2l"] = np.ascontiguousarray((np.asarray(i["r2_w"], np.float32) / 12544.0).T)
    c["r2b"] = np.asarray(i["r2_b"], np.float32).reshape(4, 1)
    c["c1bT"] = np.asarray(i["c1_b"], np.float32) * s1[None, :]
    c["bn1h64"] = h1v.reshape(64, 1)
    c["c2bT"] = np.asarray(i["c2_b"], np.float32) * s2[None, :]
    c["bn2h"] = h2v.reshape(128, 1)
    c["bn0h"] = np.tile(h0v, 4).reshape(128, 1)
    c["fcl"] = np.ascontiguousarray((np.asarray(i["fc_w"], np.float32) / 3136.0).T)
    c["fcb"] = np.asarray(i["fc_b"], np.float32).reshape(2, 1)
    return c


_PROG = None


def kernel(**inputs):
    global _PROG
    if _PROG is None:
        _PROG = build_program()
    nc = _PROG
    c = prep_consts(inputs)
    x = np.asarray(inputs["x"], np.float32).astype(NP_BF16)
    in_maps = []
    for core in range(8):
        m = dict(c)
        m["xs"] = np.ascontiguousarray(x[core * 8 : core * 8 + 8])
        in_maps.append(m)
    res = run_bass_kernel_spmd(nc, in_maps, core_ids=list(range(8)))
    out = np.concatenate([r["out"].T for r in res.results], axis=0)
    return out.astype(np.float32)
